# revision 39
# baseline (speedup 1.0000x reference)
"""MeshGraphNet on 8 Trainium2 NeuronCores (Bass/Tile, SPMD).

Strategy (see spec sharding_hint): edges partitioned across cores by
*receiver* node; node space padded 10000 -> 10240 and bin-packed into
8 cores x 10 windows x 128 nodes so each window owns <= 768 incident edges
(6 edge-tiles of 128). All matmul data is fp16 (PE: 1 cycle/row vs 4 for
fp32, with 8x finer mantissa than bf16) and flows feature-major: MLP
weights are the stationary lhsT and the activations are wide moving rhs,
so no transposes are needed inside an MLP chain. Receiver gather /
scatter-add are one-hot matmuls fused into the PSUM accumulation of the
consuming/producing MLP stage. The sender gather runs on the
W1b-transformed node embedding Z = hn @ pe_W1[128:256] + b1, AllGather'ed
to DRAM once per layer (fp16) and row-gathered by sender index via SWDGE
dma_gather in transpose mode, which lands feature-major and is accumulated
into Y1 PSUM by an identity matmul. Residual states keep fp32 masters in
SBUF (he32/hnT32) with fp16 derived copies, so state rounding does not
accumulate across the 15 layers. LayerNorm runs edge/node-major on the
vector engine (G-batched broadcast ops); relu/bias evacuations run on the
scalar engine with per-partition bias APs.
"""

import os
import sys

import numpy as np

for _p in ("/opt/trn_rl_repo", "/root/.axon_site/_ro/trn_rl_repo"):
    if os.path.isdir(_p) and _p not in sys.path:
        sys.path.insert(0, _p)

import concourse.bass as bass
import concourse.bacc as bacc
import concourse.mybir as mybir
import concourse.tile as tile
from concourse.bass_utils import run_bass_kernel_spmd

F32 = mybir.dt.float32
BF16 = mybir.dt.float16  # 16-bit compute dtype (fp16: 1 cyc/row on PE, 10-bit mantissa)
I16 = mybir.dt.int16
ALU = mybir.AluOpType
ACT_F = mybir.ActivationFunctionType

N, E, D = 10000, 60000, 128
NF, EF, NL = 12, 3, 15
CORES = 8
WPC = 10                       # windows per core
NPC = WPC * 128                # 1280 nodes per core
NPAD = CORES * NPC             # 10240
TPW = 6                        # edge tiles per window
CAP = TPW * 128                # 768 edges per window max
T = WPC * TPW                  # 60 edge tiles per core
P = T * 128                    # 7680 edge slots per core
EPS = 1e-5

NGRP = [(0, 4), (4, 3), (7, 3)]    # node window groups
EGRP = [(t0, 4) for t0 in range(0, T, 4)]   # edge-encoder tile groups


# ----------------------------------------------------------------------------
# Host-side graph packing
# ----------------------------------------------------------------------------

def pack_graph(edge_index):
    send0 = np.asarray(edge_index[0], np.int64)
    recv0 = np.asarray(edge_index[1], np.int64)
    deg = np.bincount(recv0, minlength=N)

    order = np.argsort(-deg, kind="stable")
    nwin = CORES * WPC
    win_fill = np.zeros(nwin, dtype=np.int64)    # node count per window
    win_load = np.zeros(nwin, dtype=np.int64)    # edge count per window
    perm_pos = np.full(N, -1, dtype=np.int64)
    for nid in order:
        d = deg[nid]
        cand = np.nonzero((win_fill < 128) & (win_load + d <= CAP))[0]
        assert len(cand), "window packing failed (need TPW bump)"
        w = cand[np.argmin(win_load[cand])]
        perm_pos[nid] = w * 128 + win_fill[w]
        win_fill[w] += 1
        win_load[w] += d
    assert (perm_pos >= 0).all()

    send_new = perm_pos[send0]
    recv_new = perm_pos[recv0]

    ewin = recv_new // 128
    edge_slots = np.full((CORES, P), -1, dtype=np.int64)
    for c in range(CORES):
        for wl in range(WPC):
            w = c * WPC + wl
            eids = np.nonzero(ewin == w)[0]
            base = wl * CAP
            edge_slots[c, base : base + len(eids)] = eids
    return perm_pos, send_new, recv_new, edge_slots


def _bc(v, dt=np.float32):
    """[K] -> [128, K] broadcast tile."""
    v = np.asarray(v, np.float32).reshape(1, -1)
    return np.broadcast_to(v, (128, v.shape[1])).astype(dt).copy()


def _col(v):
    """[K] -> [K, 1] fp32 column (per-partition bias/scale)."""
    return np.asarray(v, np.float32).reshape(-1, 1).copy()


def build_inputs(inp, perm_pos, send_new, recv_new, edge_slots):
    """Build in_maps (one dict per core) for the device program."""
    g32 = lambda k: np.ascontiguousarray(np.asarray(inp[k], np.float32))
    bf = lambda a: np.asarray(a, np.float32).astype(np.float16)

    nf_pad = np.zeros((NPAD, NF), np.float32)
    nf_pad[perm_pos] = g32("node_features")
    ef = g32("edge_features")

    peW1, peW2, peW3 = g32("pe_W1"), g32("pe_W2"), g32("pe_W3")
    pnW1, pnW2, pnW3 = g32("pn_W1"), g32("pn_W2"), g32("pn_W3")

    # per-layer stationary/moving weights, bf16, [NL, 128, 9*128]
    Wp = np.stack([
        np.concatenate([
            peW1[l, 0:128], peW1[l, 256:384], peW1[l, 128:256],
            peW2[l], peW3[l],
            pnW1[l, 0:128], pnW1[l, 128:256], pnW2[l], pnW3[l],
        ], axis=1)
        for l in range(NL)
    ])  # order: W1a W1c W1b W2e W3e W1n0 W1n1 W2n W3n
    # per-layer [128,128] broadcast rows: ge, zb1(edge b1), gn, b3e, b3n
    Bc = np.stack([
        np.concatenate([
            _bc(inp["pe_g"][l]), _bc(inp["pe_b1"][l]), _bc(inp["pn_g"][l]),
            _bc(inp["pe_b3"][l]), _bc(inp["pn_b3"][l]),
        ], axis=1)
        for l in range(NL)
    ])
    # per-layer fp32 columns: b2e, b1n, b2n, betae, betan
    Cc = np.stack([
        np.concatenate([
            _col(inp["pe_b2"][l]), _col(inp["pn_b1"][l]), _col(inp["pn_b2"][l]),
            _col(inp["pe_beta"][l]), _col(inp["pn_beta"][l]),
        ], axis=1)
        for l in range(NL)
    ])
    # per-layer single-partition row [1, 128]: betaW1 (deg compensation)
    betaW1 = np.stack([
        np.asarray(inp["pe_beta"][l], np.float64) @ np.asarray(pnW1[l, 128:256], np.float64)
        for l in range(NL)
    ]).astype(np.float32)
    R1 = betaW1.reshape(NL, 1, 128)

    shared = {
        "Wp": bf(Wp), "Bc": bf(Bc), "Cc": Cc, "R1": bf(R1),
        # encoders / decoder (fp16, like the layer loop)
        "encnW1": bf(g32("enc_n_W1")),
        "enceW1": bf(g32("enc_e_W1")),
        "encW": bf(np.concatenate([
            g32("enc_n_W2"), g32("enc_n_W3"),
            g32("enc_e_W2"), g32("enc_e_W3"),
            g32("dec_W1"), g32("dec_W2"),
        ], axis=1)),  # [128, 6*128]
        "decW3": bf(g32("dec_W3")),
        "encC": np.concatenate([
            _col(inp["enc_n_b1"]), _col(inp["enc_n_b2"]), _col(inp["enc_n_beta"]),
            _col(inp["enc_e_b1"]), _col(inp["enc_e_b2"]), _col(inp["enc_e_beta"]),
            _col(inp["dec_b1"]), _col(inp["dec_b2"]),
        ], axis=1),  # [128, 8] fp32
        "encG": bf(np.concatenate([
            _bc(inp["enc_n_g"]), _bc(inp["enc_e_g"]),
        ], axis=1)),  # [128, 256]
        "encR": bf(np.concatenate([
            _bc(inp["enc_n_b3"]), _bc(inp["enc_e_b3"]),
        ], axis=1)),  # [128, 256]
        "decb3bc": _bc(inp["dec_b3"]),  # [128, 3] fp32
        "ones": bf(np.ones((1, 128), np.float32)),
        "ident": bf(np.eye(128, dtype=np.float32)),
    }

    in_maps = []
    for c in range(CORES):
        sl = edge_slots[c]
        v = sl >= 0
        send_c = np.zeros(P, np.int64)
        send_c[v] = send_new[sl[v]]
        recv_c = np.zeros(P, np.int64)
        recv_c[v] = recv_new[sl[v]]

        efT = np.zeros((EF, P), np.float32)
        efT[:, v] = ef[sl[v]].T

        slots = np.arange(P)
        t, p = slots // 128, slots % 128
        j = recv_c - (c * NPC + (t // TPW) * 128)
        O = np.zeros((128, P), np.float32)
        OT = np.zeros((128, P), np.float32)
        O[p[v], t[v] * 128 + j[v]] = 1.0
        OT[j[v], t[v] * 128 + p[v]] = 1.0

        wrapped = send_c.astype(np.int16).reshape(P // 16, 16).T  # [16, P//16]
        sidx = np.tile(wrapped, (8, 1))                           # [128, P//16]

        # per-local-node incident edge count (receiver degree), [1, NPC]
        deg_c = np.zeros(NPC, np.float32)
        np.add.at(deg_c, recv_c[v] - c * NPC, 1.0)

        m = dict(shared)
        m.update({
            "nfT": bf(np.ascontiguousarray(
                nf_pad[c * NPC:(c + 1) * NPC].T).reshape(NF, WPC, 128)),
            "efT": bf(efT.reshape(EF, T, 128)),
            "O": bf(O.reshape(128, T, 128)),
            "OT": bf(OT.reshape(128, T, 128)),
            "sidx": sidx,
            "deg": bf(deg_c.reshape(1, NPC)),
        })
        in_maps.append(m)
    return in_maps


# ----------------------------------------------------------------------------
# Device program
# ----------------------------------------------------------------------------

def build_program(n_layers=NL, sim1=False):
    nc = bacc.Bacc("TRN2", target_bir_lowering=False, debug=False,
                   num_devices=1 if sim1 else CORES)

    dram = {}

    def din(name, shape, dt=BF16):
        dram[name] = nc.dram_tensor(name, list(shape), dt, kind="ExternalInput")
        return dram[name]

    din("nfT", [NF, WPC, 128])
    din("efT", [EF, T, 128])
    din("O", [128, T, 128])
    din("OT", [128, T, 128])
    din("sidx", [128, P // 16], I16)
    din("deg", [1, NPC])
    din("Wp", [NL, 128, 9 * 128])
    din("Bc", [NL, 128, 5 * 128])
    din("Cc", [NL, 128, 5], F32)
    din("R1", [NL, 1, 128])
    din("encnW1", [NF, 128])
    din("enceW1", [EF, 128])
    din("encW", [128, 6 * 128])
    din("decW3", [128, 3])
    din("encC", [128, 8], F32)
    din("encG", [128, 256])
    din("encR", [128, 256])
    din("decb3bc", [128, 3], F32)
    din("ones", [1, 128])
    din("ident", [128, 128])
    out_d = nc.dram_tensor("out", [NPC, 3], F32, kind="ExternalOutput")

    with tile.TileContext(nc) as tc:
        _build_tile_program(nc, tc, dram, out_d, n_layers, sim1)
    nc.compile()
    return nc


def _build_tile_program(nc, tc, dram, out_d, n_layers, sim1=False):
    from contextlib import ExitStack

    st = ExitStack()
    const = st.enter_context(tc.tile_pool(name="const", bufs=1))
    wpool = st.enter_context(tc.tile_pool(name="wpool", bufs=2))
    work = st.enter_context(tc.tile_pool(name="work", bufs=3))
    xpool = st.enter_context(tc.tile_pool(name="xpool", bufs=4))
    ps_mlp = st.enter_context(tc.tile_pool(name="ps_mlp", bufs=4, space="PSUM"))
    ps_tp = st.enter_context(tc.tile_pool(name="ps_tp", bufs=2, space="PSUM"))
    ps_sm = st.enter_context(tc.tile_pool(name="ps_sm", bufs=2, space="PSUM"))
    dpool = st.enter_context(tc.tile_pool(name="dram", bufs=2, space="DRAM"))

    NO_CC = bool(int(os.environ.get("K_NO_CC", "0")))
    NO_GATHER = bool(int(os.environ.get("K_NO_GATHER", "0")))
    zspace = "Local" if (sim1 or NO_CC) else "Shared"

    def mm(out, lhsT, rhs, start=True, stop=True):
        nc.tensor.matmul(out, lhsT, rhs, start=start, stop=stop)

    def cs(ap2, c):   # chunk slice: [..., K, 128*nc] -> cols of chunk c
        return ap2[:, c * 128:(c + 1) * 128]

    # ---- resident SBUF state ----
    ident = const.tile([128, 128], BF16)
    nc.sync.dma_start(ident[:], dram["ident"][:])
    ones = const.tile([1, 128], BF16)
    nc.sync.dma_start(ones[:], dram["ones"][:])
    deg = const.tile([1, NPC], BF16)
    nc.sync.dma_start(deg[:], dram["deg"][:])
    O_sb = const.tile([128, T, 128], BF16)
    nc.sync.dma_start(O_sb[:], dram["O"][:])
    OT_sb = const.tile([128, T, 128], BF16)
    nc.sync.dma_start(OT_sb[:], dram["OT"][:])
    sidx = const.tile([128, P // 16], I16)
    nc.sync.dma_start(sidx[:], dram["sidx"][:])
    eps_col = const.tile([128, 1], F32)
    nc.vector.memset(eps_col[:], EPS)
    he_fm = const.tile([128, T, 128], BF16)    # edge state, feature-major
    hnT = const.tile([128, WPC, 128], BF16)    # node state, feature-major
    he32 = const.tile([128, T, 128], F32)      # fp32 master of he_fm
    hnT32 = const.tile([128, WPC, 128], F32)   # fp32 master of hnT
    aggT = const.tile([128, WPC, 128], BF16)   # scatter result, feature-major

    # ---- encoder/decoder weights (fp16) ----
    encnW1 = const.tile([NF, 128], BF16)
    nc.sync.dma_start(encnW1[:], dram["encnW1"][:])
    enceW1 = const.tile([EF, 128], BF16)
    nc.sync.dma_start(enceW1[:], dram["enceW1"][:])
    encW = const.tile([128, 6 * 128], BF16)
    nc.sync.dma_start(encW[:], dram["encW"][:])
    decW3 = const.tile([128, 3], BF16)
    nc.sync.dma_start(decW3[:], dram["decW3"][:])
    encC = const.tile([128, 8], F32)
    nc.sync.dma_start(encC[:], dram["encC"][:])
    encG = const.tile([128, 256], BF16)
    nc.sync.dma_start(encG[:], dram["encG"][:])
    encR = const.tile([128, 256], BF16)
    nc.sync.dma_start(encR[:], dram["encR"][:])
    decb3bc = const.tile([128, 3], F32)
    nc.sync.dma_start(decb3bc[:], dram["decb3bc"][:])
    nfT = const.tile([NF, WPC, 128], BF16)
    nc.sync.dma_start(nfT[:], dram["nfT"][:])
    efT = const.tile([EF, T, 128], BF16)
    nc.sync.dma_start(efT[:], dram["efT"][:])

    def weight_tiles(l):
        Wp = wpool.tile([128, 9 * 128], BF16, tag="Wp")
        nc.sync.dma_start(Wp[:], dram["Wp"][l])
        Bc = wpool.tile([128, 5 * 128], BF16, tag="Bc")
        nc.sync.dma_start(Bc[:], dram["Bc"][l])
        Cc = wpool.tile([128, 5], F32, tag="Cc")
        nc.sync.dma_start(Cc[:], dram["Cc"][l])
        R1 = wpool.tile([1, 128], BF16, tag="R1")
        nc.sync.dma_start(R1[:], dram["R1"][l])
        return {"Wp": Wp, "Bc": Bc, "Cc": Cc, "R1": R1}

    def ln_norm(y3b, G, g_bc):
        """LN(y3b) * g, per row of [128, G, 128] (b3 already in y3b; no beta).
        Returns [128, G, 128] in dt."""
        stats = work.tile([128, G, 6], F32, tag="stats")
        mv = work.tile([128, G, 2], F32, tag="mv")
        for i in range(G):
            nc.vector.bn_stats(stats[:, i, :], y3b[:, i, :])
            nc.vector.bn_aggr(mv[:, i, :], stats[:, i, :])
        sd = work.tile([128, G, 1], F32, tag="sd")
        nc.scalar.activation(sd[:], mv[:, :, 1:2], ACT_F.Sqrt, bias=eps_col[:])
        rstd = work.tile([128, G, 1], F32, tag="rstd")
        nc.vector.reciprocal(rstd[:], sd[:])
        xng = work.tile([128, G, 128], BF16, tag="xng")
        nc.vector.tensor_tensor(
            xng[:], y3b[:], mv[:, :, 0:1].to_broadcast([128, G, 128]),
            ALU.subtract)
        nc.vector.tensor_tensor(
            xng[:], xng[:], rstd[:].to_broadcast([128, G, 128]), ALU.mult)
        nc.vector.tensor_tensor(
            xng[:], xng[:], g_bc[:, None, :].to_broadcast([128, G, 128]),
            ALU.mult)
        return xng

    def mlp_fm(rhs_slices, b1_col, W2, b2_col, W3, b3_bc, G):
        """Feature-major 3-stage MLP on G tiles. rhs_slices: list of
        (lhsT, rhs) accumulated into the Y1 PSUM. Returns y3b
        (edge/node-major, [128, G, 128] fp16, b3 added)."""
        ps1 = ps_mlp.tile([128, G, 128], F32, tag="mlp")
        nmm = len(rhs_slices)
        for i, (lt, rh) in enumerate(rhs_slices):
            mm(ps1[:], lt, rh, start=(i == 0), stop=(i == nmm - 1))
        y1 = work.tile([128, G, 128], BF16, tag="y1")
        if b1_col is None:
            nc.scalar.activation(y1[:], ps1[:], ACT_F.Relu)
        else:
            nc.scalar.activation(y1[:], ps1[:], ACT_F.Relu, bias=b1_col)
        ps2 = ps_mlp.tile([128, G, 128], F32, tag="mlp")
        mm(ps2[:], W2, y1[:])
        y2 = work.tile([128, G, 128], BF16, tag="y2")
        nc.scalar.activation(y2[:], ps2[:], ACT_F.Relu, bias=b2_col)
        ps3 = ps_mlp.tile([128, G, 128], F32, tag="mlp")
        for i in range(G):
            mm(ps3[:, i, :], y2[:, i, :], W3)
        y3b = work.tile([128, G, 128], BF16, tag="y3b")
        nc.vector.scalar_tensor_tensor(
            y3b[:], ps3[:], 0.0,
            b3_bc[:, None, :].to_broadcast([128, G, 128]),
            ALU.bypass, ALU.add)
        return y3b

    def z_alloc():
        zin = dpool.tile([NPC, 128], BF16, tag="zin")
        zout = dpool.tile([NPAD, 128], BF16, tag="zout", addr_space=zspace)
        return zin, zout

    def z_group(gi, w0, G, zin, zout, wt):
        """Z = hn @ W1b + b1e for one node group -> zin rows."""
        W1b = cs(wt["Wp"], 2)
        zb1 = cs(wt["Bc"], 1)
        for k in range(G):
            w = w0 + k
            psZ = ps_sm.tile([128, 128], F32, tag="sm")
            mm(psZ[:], hnT[:, w, :], W1b)
            z = work.tile([128, 128], BF16, tag="z")
            nc.vector.scalar_tensor_tensor(z[:], psZ[:], 0.0, zb1,
                                           ALU.bypass, ALU.add)
            nc.sync.dma_start(zin[w * 128:(w + 1) * 128, :], z[:])

    def allgather(zin, zout):
        if sim1 or NO_CC:
            for c in range(CORES):
                nc.sync.dma_start(zout[c * NPC:(c + 1) * NPC, :], zin[:])
            return
        nc.gpsimd.collective_compute(
            "AllGather", ALU.bypass,
            replica_groups=[list(range(CORES))],
            ins=[zin.opt()], outs=[zout.opt()],
        )

    # ---- encoders ----
    # node encoder: feature-major in, node-major y3, LN, transpose -> hnT
    encn_b1 = encC[:, 0:1]
    encn_b2 = encC[:, 1:2]
    encn_beta = encC[:, 2:3]
    ence_b1 = encC[:, 3:4]
    ence_b2 = encC[:, 4:5]
    ence_beta = encC[:, 5:6]
    dec_b1 = encC[:, 6:7]
    dec_b2 = encC[:, 7:8]
    wts = {0: weight_tiles(0)} if n_layers > 0 else {}
    if n_layers > 0:
        zin, zout = z_alloc()
    for gi, (w0, G) in enumerate(NGRP):
        y3b = mlp_fm([(encnW1[:], nfT[:, w0:w0 + G, :])],
                     encn_b1, cs(encW, 0), encn_b2, cs(encW, 1),
                     cs(encR, 0), G)
        xng = ln_norm(y3b, G, cs(encG, 0))
        psT = ps_tp.tile([128, G, 128], BF16, tag="tp")
        for i in range(G):
            nc.tensor.transpose(psT[:, i, :], xng[:, i, :], ident[:])
        nc.vector.tensor_scalar(hnT32[:, w0:w0 + G, :], psT[:], encn_beta,
                                None, ALU.add)
        nc.scalar.copy(hnT[:, w0:w0 + G, :], hnT32[:, w0:w0 + G, :])
        if n_layers > 0:
            z_group(gi, w0, G, zin, zout, wts[0])
    # edge encoder
    for t0, G in EGRP:
        y3b = mlp_fm([(enceW1[:], efT[:, t0:t0 + G, :])],
                     ence_b1, cs(encW, 2), ence_b2, cs(encW, 3),
                     cs(encR, 1), G)
        xng = ln_norm(y3b, G, cs(encG, 1))
        psT = ps_tp.tile([128, G, 128], BF16, tag="tp")
        for i in range(G):
            nc.tensor.transpose(psT[:, i, :], xng[:, i, :], ident[:])
        nc.vector.tensor_scalar(he32[:, t0:t0 + G, :], psT[:], ence_beta,
                                None, ALU.add)
        nc.scalar.copy(he_fm[:, t0:t0 + G, :], he32[:, t0:t0 + G, :])

    if n_layers > 0:
        allgather(zin, zout)

    # ---- message-passing layers ----
    for l in range(n_layers):
        wt = wts[l]
        Wp, Bc, Cc, R1 = wt["Wp"], wt["Bc"], wt["Cc"], wt["R1"]
        W1a, W1c = cs(Wp, 0), cs(Wp, 1)
        W2e, W3e = cs(Wp, 3), cs(Wp, 4)
        W1n0, W1n1 = cs(Wp, 5), cs(Wp, 6)
        W2n, W3n = cs(Wp, 7), cs(Wp, 8)
        ge_bc, gn_bc = cs(Bc, 0), cs(Bc, 2)
        b3e_bc, b3n_bc = cs(Bc, 3), cs(Bc, 4)
        b2e_col, b1n_col, b2n_col = Cc[:, 0:1], Cc[:, 1:2], Cc[:, 2:3]
        betae_col, betan_col = Cc[:, 3:4], Cc[:, 4:5]
        bW1_row = R1[:, 0:128]
        last = l == n_layers - 1
        if not last:
            wts[l + 1] = weight_tiles(l + 1)

        # ---- edge phase ----
        for w in range(WPC):
            # sender gather, feature-major [128, 1, CAP]
            xsT = xpool.tile([128, 1, CAP], BF16, tag="xsT")
            if NO_GATHER:
                nc.sync.dma_start(
                    xsT[:, 0, :].rearrange("p (t q) -> p t q", t=TPW),
                    zout[0:CAP, :].rearrange("(t q) p -> p t q", q=128))
            else:
                nc.gpsimd.dma_gather(
                    xsT[:], zout[:],
                    sidx[:, w * (CAP // 16):(w + 1) * (CAP // 16)],
                    CAP, CAP, 128, transpose=True,
                )
            # receiver pre-transform Rn = hn @ W1c (node-major)
            psRn = ps_sm.tile([128, 128], F32, tag="sm")
            mm(psRn[:], hnT[:, w, :], W1c)
            rn = work.tile([128, 128], BF16, tag="rn")
            nc.scalar.copy(rn[:], psRn[:])
            psA = ps_sm.tile([128, 128], F32, tag="sm")
            for g in range(2):
                t0 = w * TPW + 3 * g
                y3b = mlp_fm(
                    [(W1a, he_fm[:, t0:t0 + 3, :]),
                     (rn[:], OT_sb[:, t0:t0 + 3, :]),
                     (ident[:], xsT[:, 0, g * 384:(g + 1) * 384])],
                    None, W2e, b2e_col, W3e, b3e_bc, 3)
                xng = ln_norm(y3b, 3, ge_bc)
                # scatter-add into aggT psum (feature-major out)
                for i in range(3):
                    mm(psA[:], xng[:, i, :], O_sb[:, t0 + i, :],
                       start=(g == 0 and i == 0), stop=(g == 1 and i == 2))
                # he += (xng + beta)^T  (fp32 master, bf16 derived copy)
                psT = ps_tp.tile([128, 3, 128], BF16, tag="tp")
                for i in range(3):
                    nc.tensor.transpose(psT[:, i, :], xng[:, i, :], ident[:])
                nc.vector.scalar_tensor_tensor(
                    he32[:, t0:t0 + 3, :], psT[:], betae_col,
                    he32[:, t0:t0 + 3, :], ALU.add, ALU.add)
                nc.scalar.copy(he_fm[:, t0:t0 + 3, :], he32[:, t0:t0 + 3, :])
            nc.scalar.copy(aggT[:, w, :], psA[:])

        # ---- node phase ----
        if not last:
            zin, zout_next = z_alloc()
        for gi, (w0, G) in enumerate(NGRP):
            y3b = mlp_fm(
                [(W1n0, hnT[:, w0:w0 + G, :]),
                 (W1n1, aggT[:, w0:w0 + G, :]),
                 (bW1_row, deg[:, w0 * 128:(w0 + G) * 128])],
                b1n_col, W2n, b2n_col, W3n, b3n_bc, G)
            xng = ln_norm(y3b, G, gn_bc)
            psT = ps_tp.tile([128, G, 128], BF16, tag="tp")
            for i in range(G):
                nc.tensor.transpose(psT[:, i, :], xng[:, i, :], ident[:])
            nc.vector.scalar_tensor_tensor(
                hnT32[:, w0:w0 + G, :], psT[:], betan_col,
                hnT32[:, w0:w0 + G, :], ALU.add, ALU.add)
            nc.scalar.copy(hnT[:, w0:w0 + G, :], hnT32[:, w0:w0 + G, :])
            if not last:
                z_group(gi, w0, G, zin, zout_next, wts[l + 1])
        if not last:
            allgather(zin, zout_next)
            zout = zout_next

    # ---- decoder (fp16) ----
    for w0, G in NGRP:
        ps1 = ps_mlp.tile([128, G, 128], F32, tag="mlp")
        mm(ps1[:], cs(encW, 4), hnT[:, w0:w0 + G, :])
        d1 = work.tile([128, G, 128], BF16, tag="y1")
        nc.scalar.activation(d1[:], ps1[:], ACT_F.Relu, bias=dec_b1)
        ps2 = ps_mlp.tile([128, G, 128], F32, tag="mlp")
        mm(ps2[:], cs(encW, 5), d1[:])
        d2 = work.tile([128, G, 128], BF16, tag="y2")
        nc.scalar.activation(d2[:], ps2[:], ACT_F.Relu, bias=dec_b2)
        ps3 = ps_mlp.tile([128, G, 128], F32, tag="mlp")
        for i in range(G):
            mm(ps3[:, i, 0:3], d2[:, i, :], decW3[:])
        d3 = work.tile([128, G, 3], F32, tag="d3")
        nc.vector.scalar_tensor_tensor(
            d3[:], ps3[:, :, 0:3], 0.0,
            decb3bc[:, None, :].to_broadcast([128, G, 3]),
            ALU.bypass, ALU.add)
        nc.sync.dma_start(
            out_d[w0 * 128:(w0 + G) * 128, :].rearrange("(g p) c -> p g c", p=128),
            d3[:])

    st.close()


# ----------------------------------------------------------------------------
# Entry point
# ----------------------------------------------------------------------------

_NC_CACHE = {}


def kernel(**inputs):
    perm_pos, send_new, recv_new, edge_slots = pack_graph(inputs["edge_index"])
    in_maps = build_inputs(inputs, perm_pos, send_new, recv_new, edge_slots)

    if "nc" not in _NC_CACHE:
        _NC_CACHE["nc"] = build_program(NL)
    nc = _NC_CACHE["nc"]

    res = run_bass_kernel_spmd(nc, in_maps, list(range(CORES)))
    _NC_CACHE["last_results"] = res
    out_pad = np.concatenate([r["out"] for r in res.results], axis=0)
    return np.ascontiguousarray(out_pad[perm_pos]).astype(np.float32)


if __name__ == "__main__":
    sys.path.insert(0, "/root/problem")
    import reference
    inp = {k: np.asarray(v) for k, v in reference.setup_inputs().items()}
    got = kernel(**inp)
    exp = np.asarray(reference.reference(**inp))
    rel = np.abs(got - exp).max() / (np.abs(exp).max() + 1e-12)
    print("rel(absmax) =", rel)
